# revision 9
# baseline (speedup 1.0000x reference)
"""Sliding-window GQA attention (softcap) on 8 trn2 NeuronCores.

Problem shapes (hardcoded):
  Q [1, 32, 2048, 128] bf16, K/V [1, 8, 2048, 128] bf16 -> out [1, 32, 2048, 128] f32
  causal, window_left=256, softcap=30, scale=1/sqrt(128), GQA group=4.

Sharding: core c owns kv-head c and query heads [4c, 4c+4). Each (b, h_kv)
slice is fully independent -> no collectives.

Per-core kernel (transposed-score layout):
  For each q-head h and key-block kb (128 keys), compute the score strip
  S^T[k, q] = K_kb @ Q^T over the q-columns that kb can see:
  q in [kb*128, kb*128+384) (window_left=256 => 3 q-blocks). Softcap bounds
  scores at +-30, so softmax uses the CONSTANT shift 30 instead of a per-row
  max: p = exp(30*tanh(s/30*scale) - 30) stays in f32 range and normalizes
  identically (reference's +eps on l is a no-op in f32 since l >= e^{m-30}).
  This keeps everything in the S^T layout where the post-softmax P^T strip is
  directly the lhsT of the PV matmul -- no on-chip transpose of P is needed,
  and no partition-axis reductions anywhere. The row-sum l is obtained by
  appending a ones-column to V (column 128 of the PV matmul accumulator).
  Band masking is two 128x128 triangle multiplies per strip on DVE.
"""

import math
from contextlib import ExitStack

import numpy as np

import concourse.bacc as bacc
import concourse.bass as bass
import concourse.mybir as mybir
import concourse.tile as tile
from concourse.bass import MemorySpace
from concourse.bass_utils import run_bass_kernel_spmd

BF16 = mybir.dt.bfloat16
F32 = mybir.dt.float32

N_CORES = 8
HQ_PER_CORE = 4  # GQA group size
SQ = 2048
D = 128
NB = SQ // 128  # 16 key/query blocks
SCALE = 1.0 / math.sqrt(128.0)
SOFTCAP = 30.0

# strip widths: key-block kb sees q-columns [kb*128, kb*128 + W[kb])
WIDTHS = [min(384, SQ - kb * 128) for kb in range(NB)]
OFFS = [sum(WIDTHS[:kb]) for kb in range(NB)]
TOT = sum(WIDTHS)  # 5760 score columns per head


def build_attention(nc: bass.Bass, q, k, v, out):
    """q [4,2048,128] bf16; k,v [2048,128] bf16; out [4,2048,128] f32 (DRAM APs)."""
    with ExitStack() as ctx:
        tc = ctx.enter_context(tile.TileContext(nc))
        consts = ctx.enter_context(tc.tile_pool(name="consts", bufs=1))
        qt_pool = ctx.enter_context(tc.tile_pool(name="qt", bufs=2))
        t_pool = ctx.enter_context(tc.tile_pool(name="tbuf", bufs=2))
        p_pool = ctx.enter_context(tc.tile_pool(name="pbuf", bufs=2))
        o_pool = ctx.enter_context(tc.tile_pool(name="obuf", bufs=2))
        r_pool = ctx.enter_context(tc.tile_pool(name="rtile", bufs=4))
        spsum = ctx.enter_context(
            tc.tile_pool(name="spsum", bufs=2, space=MemorySpace.PSUM)
        )
        opsum = ctx.enter_context(
            tc.tile_pool(name="opsum", bufs=4, space=MemorySpace.PSUM)
        )

        # K^T [d, k] via DMA-transpose; V blocks + ones column.
        kt = consts.tile([128, SQ], BF16)
        nc.sync.dma_start_transpose(out=kt, in_=k)
        vt = consts.tile([128, NB, 129], BF16)
        nc.vector.memset(vt[:, :, 128:129], 1.0)
        nc.sync.dma_start(
            out=vt[:, :, 0:128], in_=v.rearrange("(t p) d -> p t d", p=128)
        )
        # band masks: strip block 0 keeps c >= kr (upper tri incl diag),
        # strip block 2 keeps c <= kr (lower tri incl diag).
        mu = consts.tile([128, 128], BF16)
        nc.gpsimd.memset(mu, 1.0)
        nc.gpsimd.affine_select(
            out=mu, in_=mu, compare_op=mybir.AluOpType.is_ge, fill=0.0,
            base=0, pattern=[[1, 128]], channel_multiplier=-1,
        )
        ml = consts.tile([128, 128], BF16)
        nc.gpsimd.memset(ml, 1.0)
        nc.gpsimd.affine_select(
            out=ml, in_=ml, compare_op=mybir.AluOpType.is_ge, fill=0.0,
            base=0, pattern=[[-1, 128]], channel_multiplier=1,
        )
        negcap = consts.tile([128, 1], F32)
        nc.vector.memset(negcap, -SOFTCAP)

        for h in range(HQ_PER_CORE):
            qt = qt_pool.tile([128, SQ], BF16)
            nc.sync.dma_start_transpose(out=qt, in_=q[h])
            tbuf = t_pool.tile([128, TOT], F32)
            pbuf = p_pool.tile([128, TOT], BF16)
            obuf = o_pool.tile([128, NB, 128], F32)

            # QK^T strips in groups of 2 (one 2-bank psum tile per group),
            # then one tanh per group reading both strips strided.
            for g in range(NB // 2):
                kb0, kb1 = 2 * g, 2 * g + 1
                sp = spsum.tile([128, 1024], F32)
                for j, kb in enumerate((kb0, kb1)):
                    w = WIDTHS[kb]
                    nc.tensor.matmul(
                        out=sp[:, j * 512 : j * 512 + w],
                        lhsT=kt[:, kb * 128 : (kb + 1) * 128],
                        rhs=qt[:, kb * 128 : kb * 128 + w],
                        start=True,
                        stop=True,
                    )
                if WIDTHS[kb0] == WIDTHS[kb1]:
                    w = WIDTHS[kb0]
                    src = sp[:].rearrange("p (g x) -> p g x", g=2)[:, :, 0:w]
                    dst = tbuf[:, OFFS[kb0] : OFFS[kb0] + 2 * w].rearrange(
                        "p (g x) -> p g x", g=2
                    )
                    nc.scalar.activation(
                        out=dst, in_=src,
                        func=mybir.ActivationFunctionType.Tanh,
                        scale=SCALE / SOFTCAP,
                    )
                else:
                    for j, kb in enumerate((kb0, kb1)):
                        w = WIDTHS[kb]
                        nc.scalar.activation(
                            out=tbuf[:, OFFS[kb] : OFFS[kb] + w],
                            in_=sp[:, j * 512 : j * 512 + w],
                            func=mybir.ActivationFunctionType.Tanh,
                            scale=SCALE / SOFTCAP,
                        )

            # p = exp(30*t - 30), bf16 (two halves so PV can start early)
            half = OFFS[NB // 2]
            for lo, hi in ((0, half), (half, TOT)):
                nc.scalar.activation(
                    out=pbuf[:, lo:hi], in_=tbuf[:, lo:hi],
                    func=mybir.ActivationFunctionType.Exp,
                    scale=SOFTCAP, bias=negcap,
                )

            # band mask: zero the invalid triangles of each strip
            for kb in range(NB):
                off = OFFS[kb]
                nc.vector.tensor_mul(
                    out=pbuf[:, off : off + 128],
                    in0=pbuf[:, off : off + 128],
                    in1=mu,
                )
                if WIDTHS[kb] == 384:
                    nc.vector.tensor_mul(
                        out=pbuf[:, off + 256 : off + 384],
                        in0=pbuf[:, off + 256 : off + 384],
                        in1=ml,
                    )

            # PV: O[qb] (+ row-sum col) accumulates over kb in {qb-2, qb-1, qb}
            otiles = {}
            for kb in range(NB):
                nq = WIDTHS[kb] // 128
                for j in range(nq):
                    qb = kb + j
                    first = kb == max(0, qb - 2)
                    if first:
                        otiles[qb] = opsum.tile(
                            [128, 129], F32, name="otile", tag="otile"
                        )
                    nc.tensor.matmul(
                        out=otiles[qb],
                        lhsT=pbuf[:, OFFS[kb] + j * 128 : OFFS[kb] + (j + 1) * 128],
                        rhs=vt[:, kb, :],
                        start=first,
                        stop=(kb == qb),
                    )
                # finalize O[kb]: divide by row-sum
                ot = otiles.pop(kb)
                r = r_pool.tile([128, 1], F32)
                nc.vector.reciprocal(out=r, in_=ot[:, 128:129])
                nc.vector.tensor_scalar_mul(
                    out=obuf[:, kb, :], in0=ot[:, 0:128], scalar1=r
                )

            nc.sync.dma_start(
                out=out[h].rearrange("(qb p) d -> p qb d", p=128), in_=obuf
            )
    return nc


_CACHED = None


def _build():
    global _CACHED
    if _CACHED is None:
        nc = bacc.Bacc()
        q = nc.dram_tensor("q", [HQ_PER_CORE, SQ, D], BF16, kind="ExternalInput")
        k = nc.dram_tensor("k", [SQ, D], BF16, kind="ExternalInput")
        v = nc.dram_tensor("v", [SQ, D], BF16, kind="ExternalInput")
        out = nc.dram_tensor("out", [HQ_PER_CORE, SQ, D], F32, kind="ExternalOutput")
        build_attention(nc, q[:], k[:], v[:], out[:])
        nc.finalize()
        _CACHED = nc
    return _CACHED


def make_in_maps(Q, K, V):
    import ml_dtypes

    Qn = np.asarray(Q).astype(ml_dtypes.bfloat16).reshape(32, SQ, D)
    Kn = np.asarray(K).astype(ml_dtypes.bfloat16).reshape(8, SQ, D)
    Vn = np.asarray(V).astype(ml_dtypes.bfloat16).reshape(8, SQ, D)
    return [
        {
            "q": np.ascontiguousarray(Qn[4 * c : 4 * c + 4]),
            "k": np.ascontiguousarray(Kn[c]),
            "v": np.ascontiguousarray(Vn[c]),
        }
        for c in range(N_CORES)
    ]


def kernel(Q, K, V):
    nc = _build()
    in_maps = make_in_maps(Q, K, V)
    res = run_bass_kernel_spmd(nc, in_maps, list(range(N_CORES))).results
    out = np.stack([res[c]["out"] for c in range(N_CORES)])  # [8,4,2048,128]
    return out.reshape(1, 32, SQ, D).astype(np.float32)
